# revision 14
# baseline (speedup 1.0000x reference)
"""Trainium2 Bass kernel for nn_ConsolidationModel.

Mathematical reduction (verified bit-exact against the reference scan):
the scan's control flow is data-independent (count depends only on t).
Consolidation fires at t=15/31/47, but between consecutive firings the
8-slot FIFO receives 4 appends + 12 shift-appends, which evicts every
consolidated row before the next firing - and after the last firing
(t=47) there are 4 appends + 11 shifts, so at t=62 the buffer holds
exactly the embeddings of tokens 55..62 with count=8.  The model output
is therefore:

    mem  = mean_p embed[seqs[:, 55+p]]          (p = 0..7)
    h    = concat([embed[query_tok], mem], -1)  (B, 128)
    out  = relu(h @ r1_w.T + r1_b) @ r2_w.T + r2_b

Device algorithm (per core, batch shard of 256 = two half-shards of 128
on the two SBUF partition halves; vocab on partitions, batch on the
free axis).  Raw bass (no TileContext) so every semaphore hop is
explicit; embed is folded into the weights on the host
(data-independent weight prep):

  toks  (128, 1156) bf16 <- ONE plain DMA (host-replicated layout, no
                            stride-0 broadcast): tail tokens batch-major
                            (cols 0:1024), query tokens (1024:1152),
                            iota (1152)
  m     = (toks == iota)  one 4x-mode DVE op -> tail masks + query
                          one-hots (128, 1152) bf16
  hist2 = reduce-sum over runs of 8 (one 2x DVE tensor_reduce)
  hidT  = A'^T @ qhot + Bm'^T @ hist   per partition-half (4 bf16
          matmuls, K=64, using PE row/col groups h0/h1)
  hid   = relu(hidT + r1_b)  one DVE tensor_scalar (add, max)
  logT  = r2wT^T @ hid       per half (2 bf16 matmuls)
  out   = logT + r2_b        one DVE tensor_scalar, then DMA out
                             (128, 512B) f32

Sharding: pure data parallel over batch across 8 cores; parameters
replicated.
"""

import numpy as np

N_CORES = 8
B = 2048           # full batch
BS = B // N_CORES  # 256 per-core batch shard
HB = BS // 2       # 128 per-half batch
H = 64             # hidden dim
V = 64             # vocab
TAIL_LO, TAIL_HI = 55, 63  # token positions that survive in the buffer
NPOS = TAIL_HI - TAIL_LO   # 8

TOKC = NPOS * HB + HB + 1  # 1024 tail + 128 query + 1 iota = 1153
TOKC_PAD = 1156            # pad to 8B alignment

_compiled_nc = None


def _build_program():
    import concourse.bacc as bacc
    import concourse.mybir as mybir

    f32 = mybir.dt.float32
    bf16 = mybir.dt.bfloat16
    eq = mybir.AluOpType.is_equal
    add = mybir.AluOpType.add
    mx = mybir.AluOpType.max

    nc = bacc.Bacc("TRN2", target_bir_lowering=False, debug=False,
                   num_devices=N_CORES)

    toks_d = nc.declare_dram_parameter("toks", [128, TOKC_PAD], bf16,
                                       isOutput=False)
    cstb_d = nc.declare_dram_parameter("cstb", [128, 192], bf16,
                                       isOutput=False)
    cstf_d = nc.declare_dram_parameter("cstf", [128, 3], f32, isOutput=False)
    out_d = nc.declare_dram_parameter("logT2", [128, HB], f32, isOutput=True)

    toks = nc.alloc_sbuf_tensor("toks_sb", [128, TOKC_PAD], bf16)
    m = nc.alloc_sbuf_tensor("m_sb", [128, NPOS * HB + HB], bf16)
    hist2 = nc.alloc_sbuf_tensor("hist2_sb", [128, HB], bf16)
    cstb = nc.alloc_sbuf_tensor("cstb_sb", [128, 192], bf16)
    cstf = nc.alloc_sbuf_tensor("cstf_sb", [128, 3], f32)
    hid = nc.alloc_sbuf_tensor("hid_sb", [128, HB], bf16)
    logsb = nc.alloc_sbuf_tensor("log_sb", [128, HB], f32)
    hid_ps = nc.alloc_psum_tensor("hid_ps", [128, HB], f32)
    log_ps = nc.alloc_psum_tensor("log_ps", [128, HB], f32)

    s_toks = nc.alloc_semaphore("s_toks")
    s_cstb = nc.alloc_semaphore("s_cstb")
    s_cstf = nc.alloc_semaphore("s_cstf")
    s_eq = nc.alloc_semaphore("s_eq")
    s_hist = nc.alloc_semaphore("s_hist")
    s_mm1 = nc.alloc_semaphore("s_mm1")
    s_relu = nc.alloc_semaphore("s_relu")
    s_mm2 = nc.alloc_semaphore("s_mm2")
    s_bias = nc.alloc_semaphore("s_bias")
    s_out = nc.alloc_semaphore("s_out")

    QT = NPOS * HB  # 1024: query one-hot column offset inside m

    # --- input DMAs: token halves split across the two HWDGE queues.
    # cstf (the eq's iota gate) goes on SP right after its token half so
    # it isn't queued behind the 49KB weight transfer.
    nc.sync.dma_start(toks[0:64, :], toks_d[0:64, :]).then_inc(s_toks, 16)
    nc.sync.dma_start(cstf[:], cstf_d[:]).then_inc(s_cstf, 16)
    nc.scalar.dma_start(toks[64:128, :], toks_d[64:128, :]).then_inc(s_toks, 16)
    nc.scalar.dma_start(cstb[:], cstb_d[:]).then_inc(s_cstb, 16)

    # --- DVE chain ---
    nc.vector.wait_ge(s_toks, 32)
    nc.vector.wait_ge(s_cstf, 16)
    # one-hot masks for 8 tail positions (batch-major) + query, one 4x op
    nc.vector.tensor_scalar(m[:], toks[:, 0:QT + HB],
                            cstf[:, 2:3], None,
                            eq).then_inc(s_eq)
    # histogram: sum the 8 masks of each batch element (runs of 8)
    with nc.allow_low_precision("histogram counts <= 8 are exact in bf16"):
        nc.vector.tensor_reduce(
            hist2[:], m[:, 0:QT].rearrange("p (b k) -> p b k", k=NPOS),
            axis=mybir.AxisListType.X, op=add).then_inc(s_hist)
    # hid = relu(hidT + r1_b): PSUM -> SBUF bf16
    nc.vector.wait_ge(s_mm1, 2)
    nc.vector.tensor_scalar(hid[:], hid_ps[:], cstf[:, 0:1], 0.0,
                            add, mx).then_inc(s_relu)
    # logits = logT + r2_b: PSUM -> SBUF f32
    nc.vector.wait_ge(s_mm2, 2)
    nc.vector.tensor_scalar(logsb[:], log_ps[:], cstf[:, 1:2], None,
                            add).then_inc(s_bias)

    # --- PE chain: 4 hid matmuls (q during the reduce), 2 logit matmuls ---
    nc.tensor.wait_ge(s_cstb, 16)
    nc.tensor.wait_ge(s_eq, 1)
    nc.tensor.matmul(hid_ps[0:64, :], cstb[0:64, 64:128], m[0:64, QT:QT + HB],
                     start=True, stop=False, skip_group_check=True)
    nc.tensor.matmul(hid_ps[64:128, :], cstb[64:128, 64:128],
                     m[64:128, QT:QT + HB],
                     start=True, stop=False, skip_group_check=True)
    nc.tensor.wait_ge(s_hist, 1)
    nc.tensor.matmul(hid_ps[0:64, :], cstb[0:64, 0:64], hist2[0:64, :],
                     start=False, stop=True,
                     skip_group_check=True).then_inc(s_mm1)
    nc.tensor.matmul(hid_ps[64:128, :], cstb[64:128, 0:64], hist2[64:128, :],
                     start=False, stop=True,
                     skip_group_check=True).then_inc(s_mm1)
    nc.tensor.wait_ge(s_relu, 1)
    nc.tensor.matmul(log_ps[0:64, :], cstb[0:64, 128:192], hid[0:64, :],
                     start=True, stop=True,
                     skip_group_check=True).then_inc(s_mm2)
    nc.tensor.matmul(log_ps[64:128, :], cstb[64:128, 128:192], hid[64:128, :],
                     start=True, stop=True,
                     skip_group_check=True).then_inc(s_mm2)

    # Output DMA, split across both queues so the dispatch instructions
    # halve in length.  No completion fence: the NEFF's fixed epilogue
    # (walrus's ~6.5us full semaphore-clear storm) runs after this and
    # dwarfs the ~1.5us the DMA needs to drain, so the output is in DRAM
    # long before the NEFF retires and the host reads it.
    nc.sync.wait_ge(s_bias, 1)
    nc.sync.dma_start(out_d[0:64, :], logsb[0:64, :]).then_inc(s_out, 16)
    nc.scalar.wait_ge(s_bias, 1)
    nc.scalar.dma_start(out_d[64:128, :], logsb[64:128, :]).then_inc(s_out, 16)

    nc.compile()
    return nc


def _prep_in_maps(inputs):
    import ml_dtypes
    bf16 = ml_dtypes.bfloat16

    embed = np.asarray(inputs["embed"], dtype=np.float32)[:V]      # (64, 64)
    r1_w = np.asarray(inputs["r1_w"], dtype=np.float32)            # (64, 128)
    r1_b = np.asarray(inputs["r1_b"], dtype=np.float32)            # (64,)
    r2_w = np.asarray(inputs["r2_w"], dtype=np.float32)            # (64, 64)
    r2_b = np.asarray(inputs["r2_b"], dtype=np.float32)            # (64,)
    seqs = np.asarray(inputs["seqs"])                              # (B, 64)
    query = np.asarray(inputs["query_tok"])                        # (B,)

    Ap = embed @ r1_w[:, :H].T                                     # (64v, 64h)
    Bm = (embed @ r1_w[:, H:].T) * np.float32(1.0 / NPOS)          # (64v, 64h)
    cstb = np.empty((128, 192), bf16)
    for half in (0, 1):
        r = slice(64 * half, 64 * half + 64)
        cstb[r, 0:64] = Bm.astype(bf16)
        cstb[r, 64:128] = Ap.astype(bf16)
        cstb[r, 128:192] = r2_w.T.astype(bf16)
    cstf = np.empty((128, 3), np.float32)
    cstf[0:64, 0] = r1_b
    cstf[64:128, 0] = r1_b
    cstf[0:64, 1] = r2_b
    cstf[64:128, 1] = r2_b
    cstf[0:64, 2] = np.arange(64, dtype=np.float32)
    cstf[64:128, 2] = np.arange(64, dtype=np.float32)

    # token layout: per half-shard of 128 batch rows, tail tokens
    # batch-major (col b*8+k), then query tokens, then the iota column;
    # replicated across the 64 vocab partitions of that half.
    tail = seqs[:, TAIL_LO:TAIL_HI].astype(np.float32)             # (B, 8)
    qf = query.astype(np.float32)                                  # (B,)
    toks = np.zeros((N_CORES, 128, TOKC_PAD), np.float32)
    QT = NPOS * HB
    for c in range(N_CORES):
        for half in (0, 1):
            base = c * BS + half * HB
            rows = slice(64 * half, 64 * half + 64)
            toks[c, rows, 0:QT] = tail[base:base + HB].reshape(1, QT)
            toks[c, rows, QT:QT + HB] = qf[base:base + HB].reshape(1, HB)
            toks[c, rows, QT + HB] = np.arange(64, dtype=np.float32)[:, None][:, 0]
    toks = toks.astype(bf16)

    return [
        {"toks": toks[c], "cstb": cstb, "cstf": cstf}
        for c in range(N_CORES)
    ]


def kernel(**inputs):
    global _compiled_nc
    from concourse.bass_utils import run_bass_kernel_spmd

    in_maps = _prep_in_maps(inputs)
    if _compiled_nc is None:
        _compiled_nc = _build_program()
    res = run_bass_kernel_spmd(_compiled_nc, in_maps, list(range(N_CORES)))
    out = np.empty((B, V), np.float32)
    for c in range(N_CORES):
        r = np.asarray(res.results[c]["logT2"], dtype=np.float32)  # (128, 128)
        out[c * BS:c * BS + HB] = r[0:64].T
        out[c * BS + HB:c * BS + BS] = r[64:128].T
    return out


if __name__ == "__main__":
    rng = np.random.default_rng(0)
    demo = {
        "embed": rng.standard_normal((V + 2, H)).astype(np.float32),
        "r1_w": rng.standard_normal((H, 2 * H)).astype(np.float32) * 0.05,
        "r1_b": rng.standard_normal(H).astype(np.float32) * 0.02,
        "r2_w": rng.standard_normal((V, H)).astype(np.float32) * 0.05,
        "r2_b": rng.standard_normal(V).astype(np.float32) * 0.02,
        "seqs": rng.integers(0, V, (B, 64)),
        "query_tok": rng.integers(0, V, (B,)),
    }
    out = kernel(**demo)
    tail = demo["embed"][demo["seqs"][:, TAIL_LO:TAIL_HI]]
    mem = tail.sum(1) / NPOS
    h = np.concatenate([demo["embed"][demo["query_tok"]], mem], -1)
    exp = np.maximum(h @ demo["r1_w"].T + demo["r1_b"], 0) @ demo["r2_w"].T + demo["r2_b"]
    err = np.abs(out - exp).max() / np.abs(exp).max()
    print("self-check rel err:", err)


# revision 15
# speedup vs baseline: 1.0186x; 1.0186x over previous
"""Trainium2 Bass kernel for nn_ConsolidationModel.

Mathematical reduction (verified bit-exact against the reference scan):
the scan's control flow is data-independent (count depends only on t).
Consolidation fires at t=15/31/47, but between consecutive firings the
8-slot FIFO receives 4 appends + 12 shift-appends, which evicts every
consolidated row before the next firing - and after the last firing
(t=47) there are 4 appends + 11 shifts, so at t=62 the buffer holds
exactly the embeddings of tokens 55..62 with count=8.  The model output
is therefore:

    mem  = mean_p embed[seqs[:, 55+p]]          (p = 0..7)
    h    = concat([embed[query_tok], mem], -1)  (B, 128)
    out  = relu(h @ r1_w.T + r1_b) @ r2_w.T + r2_b

Device algorithm (per core, batch shard of 256 = two half-shards of 128
on the two SBUF partition halves; vocab on partitions, batch on the
free axis).  Raw bass (no TileContext) so every semaphore hop is
explicit; embed is folded into the weights on the host
(data-independent weight prep):

  toks  (128, 1156) bf16 <- ONE plain DMA (host-replicated layout, no
                            stride-0 broadcast): tail tokens batch-major
                            (cols 0:1024), query tokens (1024:1152),
                            iota (1152)
  m     = (toks == iota)  one 4x-mode DVE op -> tail masks + query
                          one-hots (128, 1152) bf16
  hist2 = reduce-sum over runs of 8 (one 2x DVE tensor_reduce)
  hidT  = A'^T @ qhot + Bm'^T @ hist   per partition-half (4 bf16
          matmuls, K=64, using PE row/col groups h0/h1)
  hid   = relu(hidT + r1_b)  one DVE tensor_scalar (add, max)
  logT  = r2wT^T @ hid       per half (2 bf16 matmuls)
  out   = logT + r2_b        one DVE tensor_scalar, then DMA out
                             (128, 512B) f32

Sharding: pure data parallel over batch across 8 cores; parameters
replicated.
"""

import numpy as np

N_CORES = 8
B = 2048           # full batch
BS = B // N_CORES  # 256 per-core batch shard
HB = BS // 2       # 128 per-half batch
H = 64             # hidden dim
V = 64             # vocab
TAIL_LO, TAIL_HI = 55, 63  # token positions that survive in the buffer
NPOS = TAIL_HI - TAIL_LO   # 8

TOKC = NPOS * HB + HB + 1  # 1024 tail + 128 query + 1 iota = 1153
TOKC_PAD = 1156            # pad to 8B alignment

_compiled_nc = None


def _build_program():
    import concourse.bacc as bacc
    import concourse.mybir as mybir

    f32 = mybir.dt.float32
    bf16 = mybir.dt.bfloat16
    eq = mybir.AluOpType.is_equal
    add = mybir.AluOpType.add
    mx = mybir.AluOpType.max

    nc = bacc.Bacc("TRN2", target_bir_lowering=False, debug=False,
                   num_devices=N_CORES)

    toks_d = nc.declare_dram_parameter("toks", [128, TOKC_PAD], bf16,
                                       isOutput=False)
    cstb_d = nc.declare_dram_parameter("cstb", [128, 192], bf16,
                                       isOutput=False)
    cstf_d = nc.declare_dram_parameter("cstf", [128, 3], f32, isOutput=False)
    out_d = nc.declare_dram_parameter("logT2", [128, HB], f32, isOutput=True)

    toks = nc.alloc_sbuf_tensor("toks_sb", [128, TOKC_PAD], bf16)
    m = nc.alloc_sbuf_tensor("m_sb", [128, NPOS * HB + HB], bf16)
    hist2 = nc.alloc_sbuf_tensor("hist2_sb", [128, HB], bf16)
    cstb = nc.alloc_sbuf_tensor("cstb_sb", [128, 192], bf16)
    cstf = nc.alloc_sbuf_tensor("cstf_sb", [128, 3], f32)
    hid = nc.alloc_sbuf_tensor("hid_sb", [128, HB], bf16)
    logsb = nc.alloc_sbuf_tensor("log_sb", [128, HB], f32)
    hid_ps = nc.alloc_psum_tensor("hid_ps", [128, HB], f32)
    log_ps = nc.alloc_psum_tensor("log_ps", [128, HB], f32)

    s_toks = nc.alloc_semaphore("s_toks")
    s_cstb = nc.alloc_semaphore("s_cstb")
    s_cstf = nc.alloc_semaphore("s_cstf")
    s_eq = nc.alloc_semaphore("s_eq")
    s_hist = nc.alloc_semaphore("s_hist")
    s_mm1 = nc.alloc_semaphore("s_mm1")
    s_relu = nc.alloc_semaphore("s_relu")
    s_mm2 = nc.alloc_semaphore("s_mm2")
    s_bias = nc.alloc_semaphore("s_bias")
    s_out = nc.alloc_semaphore("s_out")

    QT = NPOS * HB  # 1024: query one-hot column offset inside m

    # --- input DMAs: token halves split across the two HWDGE queues.
    # cstf (the eq's iota gate) goes on SP right after its token half so
    # it isn't queued behind the 49KB weight transfer.
    nc.sync.dma_start(toks[0:64, :], toks_d[0:64, :]).then_inc(s_toks, 16)
    nc.sync.dma_start(cstf[:], cstf_d[:]).then_inc(s_cstf, 16)
    nc.scalar.dma_start(toks[64:128, :], toks_d[64:128, :]).then_inc(s_toks, 16)
    nc.scalar.dma_start(cstb[:], cstb_d[:]).then_inc(s_cstb, 16)

    # --- DVE chain ---
    nc.vector.wait_ge(s_toks, 32)
    nc.vector.wait_ge(s_cstf, 16)
    # one-hot masks for 8 tail positions (batch-major) + query, one 4x op
    nc.vector.tensor_scalar(m[:], toks[:, 0:QT + HB],
                            cstf[:, 2:3], None,
                            eq).then_inc(s_eq)
    # histogram: sum the 8 masks of each batch element (runs of 8)
    with nc.allow_low_precision("histogram counts <= 8 are exact in bf16"):
        nc.vector.tensor_reduce(
            hist2[:], m[:, 0:QT].rearrange("p (b k) -> p b k", k=NPOS),
            axis=mybir.AxisListType.X, op=add).then_inc(s_hist)
    # hid = relu(hidT + r1_b): PSUM -> SBUF bf16
    nc.vector.wait_ge(s_mm1, 2)
    nc.vector.tensor_scalar(hid[:], hid_ps[:], cstf[:, 0:1], 0.0,
                            add, mx).then_inc(s_relu)
    # logits = logT + r2_b: PSUM -> SBUF f32
    nc.vector.wait_ge(s_mm2, 2)
    nc.vector.tensor_scalar(logsb[:], log_ps[:], cstf[:, 1:2], None,
                            add).then_inc(s_bias)

    # --- PE chain: 4 hid matmuls (q during the reduce), 2 logit matmuls ---
    nc.tensor.wait_ge(s_cstb, 16)
    nc.tensor.wait_ge(s_eq, 1)
    nc.tensor.matmul(hid_ps[0:64, :], cstb[0:64, 64:128], m[0:64, QT:QT + HB],
                     start=True, stop=False, skip_group_check=True)
    nc.tensor.matmul(hid_ps[64:128, :], cstb[64:128, 64:128],
                     m[64:128, QT:QT + HB],
                     start=True, stop=False, skip_group_check=True)
    nc.tensor.wait_ge(s_hist, 1)
    nc.tensor.matmul(hid_ps[0:64, :], cstb[0:64, 0:64], hist2[0:64, :],
                     start=False, stop=True,
                     skip_group_check=True).then_inc(s_mm1)
    nc.tensor.matmul(hid_ps[64:128, :], cstb[64:128, 0:64], hist2[64:128, :],
                     start=False, stop=True,
                     skip_group_check=True).then_inc(s_mm1)
    nc.tensor.wait_ge(s_relu, 1)
    nc.tensor.matmul(log_ps[0:64, :], cstb[0:64, 128:192], hid[0:64, :],
                     start=True, stop=True,
                     skip_group_check=True).then_inc(s_mm2)
    nc.tensor.matmul(log_ps[64:128, :], cstb[64:128, 128:192], hid[64:128, :],
                     start=True, stop=True,
                     skip_group_check=True).then_inc(s_mm2)

    # Output DMA.  No completion fence: the NEFF's fixed epilogue
    # (walrus's ~6.5us full semaphore-clear storm) runs after this and
    # dwarfs the ~1.5us the DMA needs to drain, so the output is in DRAM
    # long before the NEFF retires and the host reads it.
    nc.sync.wait_ge(s_bias, 1)
    nc.sync.dma_start(out_d[:], logsb[:]).then_inc(s_out, 16)

    nc.compile()
    return nc


def _prep_in_maps(inputs):
    import ml_dtypes
    bf16 = ml_dtypes.bfloat16

    embed = np.asarray(inputs["embed"], dtype=np.float32)[:V]      # (64, 64)
    r1_w = np.asarray(inputs["r1_w"], dtype=np.float32)            # (64, 128)
    r1_b = np.asarray(inputs["r1_b"], dtype=np.float32)            # (64,)
    r2_w = np.asarray(inputs["r2_w"], dtype=np.float32)            # (64, 64)
    r2_b = np.asarray(inputs["r2_b"], dtype=np.float32)            # (64,)
    seqs = np.asarray(inputs["seqs"])                              # (B, 64)
    query = np.asarray(inputs["query_tok"])                        # (B,)

    Ap = embed @ r1_w[:, :H].T                                     # (64v, 64h)
    Bm = (embed @ r1_w[:, H:].T) * np.float32(1.0 / NPOS)          # (64v, 64h)
    cstb = np.empty((128, 192), bf16)
    for half in (0, 1):
        r = slice(64 * half, 64 * half + 64)
        cstb[r, 0:64] = Bm.astype(bf16)
        cstb[r, 64:128] = Ap.astype(bf16)
        cstb[r, 128:192] = r2_w.T.astype(bf16)
    cstf = np.empty((128, 3), np.float32)
    cstf[0:64, 0] = r1_b
    cstf[64:128, 0] = r1_b
    cstf[0:64, 1] = r2_b
    cstf[64:128, 1] = r2_b
    cstf[0:64, 2] = np.arange(64, dtype=np.float32)
    cstf[64:128, 2] = np.arange(64, dtype=np.float32)

    # token layout: per half-shard of 128 batch rows, tail tokens
    # batch-major (col b*8+k), then query tokens, then the iota column;
    # replicated across the 64 vocab partitions of that half.
    tail = seqs[:, TAIL_LO:TAIL_HI].astype(np.float32)             # (B, 8)
    qf = query.astype(np.float32)                                  # (B,)
    toks = np.zeros((N_CORES, 128, TOKC_PAD), np.float32)
    QT = NPOS * HB
    for c in range(N_CORES):
        for half in (0, 1):
            base = c * BS + half * HB
            rows = slice(64 * half, 64 * half + 64)
            toks[c, rows, 0:QT] = tail[base:base + HB].reshape(1, QT)
            toks[c, rows, QT:QT + HB] = qf[base:base + HB].reshape(1, HB)
            toks[c, rows, QT + HB] = np.arange(64, dtype=np.float32)[:, None][:, 0]
    toks = toks.astype(bf16)

    return [
        {"toks": toks[c], "cstb": cstb, "cstf": cstf}
        for c in range(N_CORES)
    ]


def kernel(**inputs):
    global _compiled_nc
    from concourse.bass_utils import run_bass_kernel_spmd

    in_maps = _prep_in_maps(inputs)
    if _compiled_nc is None:
        _compiled_nc = _build_program()
    res = run_bass_kernel_spmd(_compiled_nc, in_maps, list(range(N_CORES)))
    out = np.empty((B, V), np.float32)
    for c in range(N_CORES):
        r = np.asarray(res.results[c]["logT2"], dtype=np.float32)  # (128, 128)
        out[c * BS:c * BS + HB] = r[0:64].T
        out[c * BS + HB:c * BS + BS] = r[64:128].T
    return out


if __name__ == "__main__":
    rng = np.random.default_rng(0)
    demo = {
        "embed": rng.standard_normal((V + 2, H)).astype(np.float32),
        "r1_w": rng.standard_normal((H, 2 * H)).astype(np.float32) * 0.05,
        "r1_b": rng.standard_normal(H).astype(np.float32) * 0.02,
        "r2_w": rng.standard_normal((V, H)).astype(np.float32) * 0.05,
        "r2_b": rng.standard_normal(V).astype(np.float32) * 0.02,
        "seqs": rng.integers(0, V, (B, 64)),
        "query_tok": rng.integers(0, V, (B,)),
    }
    out = kernel(**demo)
    tail = demo["embed"][demo["seqs"][:, TAIL_LO:TAIL_HI]]
    mem = tail.sum(1) / NPOS
    h = np.concatenate([demo["embed"][demo["query_tok"]], mem], -1)
    exp = np.maximum(h @ demo["r1_w"].T + demo["r1_b"], 0) @ demo["r2_w"].T + demo["r2_b"]
    err = np.abs(out - exp).max() / np.abs(exp).max()
    print("self-check rel err:", err)
